# revision 47
# baseline (speedup 1.0000x reference)
"""BCM_Conv2d_fft kernel for Trainium2 (8 NeuronCores, batch-parallel).

The reference is a block-circulant 3x3 conv computed via per-block
rfft/irfft over the channel-block axis (block size 8). Per-frequency the
block products are independent, so in a real-DFT channel basis the
256->256 channel mixing matrix of each conv tap is block-diagonal with
frequency groups {f0:32, f4:32, f1:64, f2:64, f3:64}. Grouping
{f0,f4,f1} -> chunk0 and {f2,f3} -> chunk1 makes every tap's mixing
matrix chunk-diagonal: the conv needs 9 matmuls per output tile per
chunk instead of 18 - half the direct-conv PE work.

Device pipeline per core (one image):
  1. fwd:  xhat = A @ x       per pixel (A = real-DFT, freq-major rows)
  2. conv: ohat = sum_pos M_pos @ shift(xhat)   (chunk-diagonal M)
  3. inv:  out  = Ainv @ ohat

All device tensors are fp16 (halves HBM traffic vs fp32, full PE rate);
PSUM accumulation is fp32. The bias add and the fp16 -> fp32 output
conversion happen on host (the pipeline is linear, so this is exact
modulo fp16 rounding of the output).

Schedule notes (v2):
  - Short PE warm-up (3 x 256-col matmuls) fills only the DMA latency
    window; the real fwd/conv stream continues the HAM clock ramp.
  - DMA: two HWDGE rings (sync + scalar), FIFO per ring, ordered by
    first-need. Ring bytes are balanced (~1.3 MB each): sync carries x
    chunk0 + early x chunk1 rows; scalar carries the weights (packed
    c-major so conv chunk0's taps arrive before chunk1's) + late x
    chunk1 rows.
  - psum->sbuf casts are split across engines: fwd+conv on DVE, inv on
    the ACT (scalar) engine, so neither engine's copy load exceeds the
    PE window.
  - The last conv tile is split into 2x4 rows to shorten the drain.
Sharding: batch B=8 -> one image per core.
"""

import os

import numpy as np

import concourse.bacc as bacc
import concourse.mybir as mybir
import concourse.tile as tile
from concourse.bass import ts
from concourse.bass_utils import run_bass_kernel_spmd

N_CORES = 8
C = 256
H = W = 56
HP = H + 2
KK = 3
BS = 8
L = H * W                # 3136 output pixels
LP = HP * HP             # 3364 padded pixels
RPT = 8                  # output rows per conv tile
NT = RPT * W             # 448 pixels per conv tile
N_TILES = L // NT        # 7
# x ships interior-only ([128, 56*56] per chunk); fwd transforms
# interior pixels and its casts scatter into the padded xhat layout
# (rows stride 58, cols 1..57). xhat's pad rows/cols are memset once.
# fwd tile edges are row-aligned (multiples of 56) and match the x DMA
# piece boundaries so each fwd tile waits on exactly one piece.
LI = H * W               # 3136 interior pixels per chunk
FWD_EDGES = [0, 280, 616, 1120, 1456, 1960, 2464, 2968, LI]
N_FWD = len(FWD_EDGES) - 1
# emit fwd tile k after conv tile nt (dependency-safe lead)
FWD_LEAD = {0: [2], 1: [3, 4], 2: [5], 3: [6], 4: [7]}
# scheduler-sim start floors (ms units = us/1000; sim t0 ~= real 7us):
# keep each fwd tile from being statically scheduled before its x piece
# can really arrive, so the PE never parks on a data-starved group.
FWD_STAMP = {2: 0.0055, 3: 0.0058, 4: 0.0085, 5: 0.0087, 6: 0.0095,
             7: 0.0097}
MCH = C // 128           # 2 channel chunks

F32 = mybir.dt.float32
F16 = mybir.dt.float16
F8 = mybir.dt.float8e3          # e3m4: 4 mantissa bits
CONV_WT_SCALE = 16.0            # pow2: lossless headroom shift for e3m4

# fwd+inv weights ride a fp16 tensor [128, 8*128]; the 18 conv tap
# blocks ride a separate fp8-e3m4 tensor [128, 18*128] (halves the
# dominant weight stream; mixed fp8-lhsT x fp16-rhs matmul runs at full
# rate). conv blocks are c-major: chunk0's taps stream in before
# chunk1's.
FWD_BLK = lambda i, c: c * MCH + i            # i = in chunk, c = out chunk
INV_BLK = lambda k, m: 4 + k * MCH + m
CONV_BLK = lambda pos, c: c * (KK * KK) + pos
N_BLKS16 = 8
N_BLKS8 = 18

LAST_RESULT = None


def _freq_matrices(w: np.ndarray):
    """Build A [256,256], Ms (9x [256,256] chunk-diag), Ainv from w."""
    F = np.zeros((8, 8))
    FI = np.fft.rfft(np.eye(8), axis=-1)
    F[0] = FI[:, 0].real
    F[1], F[2] = FI[:, 1].real, FI[:, 1].imag
    F[3], F[4] = FI[:, 2].real, FI[:, 2].imag
    F[5], F[6] = FI[:, 3].real, FI[:, 3].imag
    F[7] = FI[:, 4].real

    def fm(bk, comp):
        if comp == 0:
            return bk
        if comp == 7:
            return 32 + bk
        if comp in (1, 2):
            return 64 + 2 * bk + (comp - 1)
        if comp in (3, 4):
            return 128 + 2 * bk + (comp - 3)
        return 192 + 2 * bk + (comp - 5)

    A = np.zeros((256, 256))
    for bk in range(32):
        for comp in range(8):
            A[fm(bk, comp), bk * 8:(bk + 1) * 8] = F[comp]
    Ainv = np.linalg.inv(A)

    wf = np.fft.rfft(w.astype(np.float64), axis=-1)  # [32, 288, 5]
    Ms = []
    for pos in range(9):
        M = np.zeros((256, 256))
        for pb in range(32):
            for kb in range(32):
                kc = pos * 32 + kb
                M[fm(pb, 0), fm(kb, 0)] += wf[pb, kc, 0].real
                M[fm(pb, 7), fm(kb, 7)] += wf[pb, kc, 4].real
                for fi in range(3):
                    re_i, im_i = 1 + 2 * fi, 2 + 2 * fi
                    Wr, Wi = wf[pb, kc, fi + 1].real, wf[pb, kc, fi + 1].imag
                    M[fm(pb, re_i), fm(kb, re_i)] += Wr
                    M[fm(pb, re_i), fm(kb, im_i)] += -Wi
                    M[fm(pb, im_i), fm(kb, re_i)] += Wi
                    M[fm(pb, im_i), fm(kb, im_i)] += Wr
        Ms.append(M)
    return A, Ms, Ainv


def _pack_weights(w: np.ndarray):
    """-> (wts16 [128,8*128] fp16, wts8 [128,18*128] e3m4) lhsT blocks."""
    import ml_dtypes

    A, Ms, Ainv = _freq_matrices(w)
    wts16 = np.zeros((128, N_BLKS16 * 128), np.float16)
    wts8 = np.zeros((128, N_BLKS8 * 128), ml_dtypes.float8_e3m4)

    sl = lambda i: slice(i * 128, (i + 1) * 128)
    for i in range(MCH):
        for c in range(MCH):
            wts16[:, sl(FWD_BLK(i, c))] = A[sl(c), sl(i)].T.astype(np.float16)
    for k in range(MCH):
        for m in range(MCH):
            wts16[:, sl(INV_BLK(k, m))] = Ainv[sl(m), sl(k)].T.astype(
                np.float16)
    for pos in range(9):
        for c in range(MCH):
            wts8[:, sl(CONV_BLK(pos, c))] = (
                Ms[pos][sl(c), sl(c)].T * CONV_WT_SCALE
            ).astype(ml_dtypes.float8_e3m4)
    return wts16, wts8


def _kernel_body(tc, x, wts16, wts8, out):
    nc = tc.nc
    with (
        tc.tile_pool(name="const", bufs=1) as const_pool,
        tc.tile_pool(name="xp", bufs=1) as xp_pool,
        tc.tile_pool(name="xh", bufs=1) as xh_pool,
        tc.tile_pool(name="oh", bufs=6) as oh_pool,
        tc.tile_pool(name="ob", bufs=4) as ob_pool,
        tc.tile_pool(name="psf", bufs=3, space="PSUM") as psf_pool,
        tc.tile_pool(name="psc", bufs=3, space="PSUM") as psc_pool,
        tc.tile_pool(name="psi", bufs=2, space="PSUM") as psi_pool,
    ):
        wt16_sb = const_pool.tile([128, N_BLKS16 * 128], F16)
        wt8_sb = const_pool.tile([128, N_BLKS8 * 128], F8)
        blk16 = lambda idx: wt16_sb[:, ts(idx, 128)]
        blk8 = lambda idx: wt8_sb[:, ts(idx, 128)]
        xq = [xp_pool.tile([128, LI], F16, tag=f"xp{i}", name=f"xq{i}")
              for i in range(MCH)]
        xhat = [xh_pool.tile([128, LP], F16, tag=f"xh{c}", name=f"xhat{c}")
                for c in range(MCH)]

        def x_dma(eng, i, r0, r1):  # r0, r1 in interior rows (0..56)
            eng.dma_start(
                out=xq[i][:, r0 * W:r1 * W], in_=x[i, :, r0 * W:r1 * W]
            )

        def w16_dma(eng, b0, b1):
            eng.dma_start(out=wt16_sb[:, b0 * 128:b1 * 128],
                          in_=wts16[:, b0 * 128:b1 * 128])

        def w8_dma(eng, b0, b1):
            eng.dma_start(out=wt8_sb[:, b0 * 128:b1 * 128],
                          in_=wts8[:, b0 * 128:b1 * 128])

        # PE warm-up: the HAM clock gate lifts the 1.2GHz cold throttle
        # only after ~3.4us of UNINTERRUPTED PE activity (any idle gap,
        # even ~150ns, resets the accumulator). Bridge from program
        # start until the first input pieces land (~9.7us) with a
        # continuous dummy stream so the lift (~10.9us) happens during
        # the first real tiles and the stream never stalls.
        warm = const_pool.tile([128, 512], F16, name="warm")
        nc.gpsimd.memset(warm[:], 0.0)
        for _ in range(10):
            ps = psf_pool.tile([128, 512], F32, tag="psf", name="wps")
            nc.tensor.matmul(ps[:], lhsT=warm[:, 0:128], rhs=warm[:],
                             start=True, stop=True)

        # zero xhat's pad border (fwd only writes interior pixels)
        for c in range(MCH):
            xv = xhat[c][:].rearrange("p (h w) -> p h w", h=HP)
            nc.gpsimd.memset(xhat[c][:, 0:59], 0.0)        # pad row 0
            nc.gpsimd.memset(xhat[c][:, 3305:LP], 0.0)     # pad row 57
            nc.gpsimd.memset(xv[:, 1:57, 0], 0.0)          # pad col 0
            nc.gpsimd.memset(xv[:, 1:57, 57], 0.0)         # pad col 57

        # Input DMA: both rings start with the pieces fwd tile 0 needs
        # (x rows 1-6 of both chunks - rows 0/57 are pad, never read -
        # plus the fwd weight blocks), then stream in first-need order.
        # Piece completion = slowest of the 16 SDMA engines (~1us skew),
        # so fwd tile edges align with piece boundaries.
        x_dma(nc.sync, 0, 0, 11)       # 157 KB - fwd tiles 0-1 chunk0
        w16_dma(nc.scalar, 0, 2)       # 66 KB  - fwd weights, out chunk0
        x_dma(nc.sync, 1, 0, 11)       # 157 KB - fwd tiles 0-1 chunk1
        w16_dma(nc.scalar, 2, 4)       # 66 KB  - fwd weights, out chunk1
        w8_dma(nc.scalar, 0, 5)        # 82 KB  - conv taps 0-4, chunk0
        x_dma(nc.sync, 0, 11, 26)      # 215 KB
        w8_dma(nc.scalar, 5, 9)        # 66 KB  - conv taps 5-8, chunk0
        x_dma(nc.sync, 1, 11, 26)      # 215 KB
        w8_dma(nc.scalar, 9, 14)       # 82 KB  - conv taps 0-4, chunk1
        w8_dma(nc.scalar, 14, 18)      # 66 KB  - conv taps 5-8, chunk1
        w16_dma(nc.scalar, 4, 8)       # 131 KB - inv weights
        x_dma(nc.sync, 0, 26, 44)      # 258 KB
        x_dma(nc.scalar, 1, 26, 44)    # 258 KB
        x_dma(nc.scalar, 0, 44, 56)    # 172 KB
        x_dma(nc.scalar, 1, 44, 56)    # 172 KB

        def fwd_tile(k):
            """Freq-transform interior pixels [FWD_EDGES[k], FWD_EDGES[k+1]).

            Edges are row-aligned; the cast scatters the rows into the
            padded xhat layout (row stride 58, cols 1..57).
            """
            p0, p1 = FWD_EDGES[k], FWD_EDGES[k + 1]
            npx = p1 - p0
            r0, r1 = p0 // W, p1 // W
            for c in range(MCH):
                ps = psf_pool.tile([128, 512], F32, tag="psf")
                for i in range(MCH):
                    nc.tensor.matmul(
                        ps[:, :npx], lhsT=blk16(FWD_BLK(i, c)),
                        rhs=xq[i][:, p0:p0 + npx],
                        start=(i == 0), stop=(i == MCH - 1),
                    )
                psv = ps[:, :npx].rearrange("p (h w) -> p h w", w=W)
                xv = xhat[c][:].rearrange("p (h w) -> p h w", h=HP)
                dst = xv[:, r0 + 1:r1 + 1, 1:57]
                # chunk casts run on different engines in parallel so a
                # conv tile's chunk0 pass isn't gated on chunk1's cast
                if c == 0:
                    nc.vector.tensor_copy(dst, psv)
                else:
                    nc.scalar.copy(dst, psv)

        # out viewed as [p(128), m(2), pix]: out channel = m*128 + p
        def conv_inv(px0, npx, ob, ob_off, ship, par_tail=False):
            """Freq conv + inverse transform for pixels [px0, px0+npx).

            px0 must be a multiple of W; npx a multiple of W. ship: list
            of (engine, m) output DMAs to issue right after chunk m's
            cast lands (empty -> caller ships the whole ob later).
            """
            r0 = px0 // W
            nr = npx // W
            ohat = []
            for c in range(MCH):
                ps = psc_pool.tile([128, NT], F32, tag="psc")
                n_mm = 0
                for kh in range(KK):
                    for kw in range(KK):
                        pos = kh * KK + kw
                        xhv = xhat[c][:].rearrange("p (h w) -> p h w", h=HP)
                        rhs = xhv[:, r0 + kh: r0 + kh + nr, kw: kw + W]
                        nc.tensor.matmul(
                            ps[:, :npx], lhsT=blk8(CONV_BLK(pos, c)), rhs=rhs,
                            start=(n_mm == 0), stop=(n_mm == KK * KK - 1),
                        )
                        n_mm += 1
                oh = oh_pool.tile([128, NT], F16, tag="oh")
                nc.vector.tensor_copy(oh[:, :npx], ps[:, :npx])
                ohat.append(oh)
            for m in range(MCH):
                ps = psi_pool.tile([128, NT], F32, tag="psi")
                for k in range(MCH):
                    nc.tensor.matmul(
                        ps[:, :npx], lhsT=blk16(INV_BLK(k, m)),
                        rhs=ohat[k][:, :npx],
                        start=(k == 0), stop=(k == MCH - 1),
                    )
                # inv casts ride the ACT engine (DVE carries fwd+conv);
                # the final half-tile casts its chunks on both engines
                # in parallel to shorten the drain
                if par_tail and m == 1:
                    nc.vector.tensor_copy(ob[:, m, ob_off:ob_off + npx],
                                          ps[:, :npx])
                else:
                    nc.scalar.copy(ob[:, m, ob_off:ob_off + npx],
                                   ps[:, :npx])
                for eng, mm in ship:
                    if mm == m:
                        eng.dma_start(
                            out=out[:, m, px0:px0 + npx],
                            in_=ob[:, m, ob_off:ob_off + npx],
                        )

        # conv tile nt reads xhat pixels [464nt, 464nt+638). Later fwd
        # tiles are emitted AFTER the conv tile that precedes their
        # first consumer, with scheduler-sim start floors so the static
        # schedule never parks the PE on a fwd group whose x piece
        # hasn't really arrived yet.
        for k in range(2):
            fwd_tile(k)
        for nt in range(N_TILES):
            ob = ob_pool.tile([128, MCH, NT], F16, tag="ob")
            if nt < N_TILES - 1:
                conv_inv(nt * NT, NT, ob, 0, ship=[])
                eng = nc.scalar if nt % 2 == 0 else nc.sync
                eng.dma_start(out=out[:, :, ts(nt, NT)], in_=ob[:])
            else:
                # tail: a half-tile then two quarter-tiles, each chunk
                # shipped as its cast lands, smallest transfers last to
                # minimize the final drain chain
                half, q = NT // 2, NT // 4
                conv_inv(nt * NT, half, ob, 0,
                         ship=[(nc.scalar, 0), (nc.sync, 1)])
                ob2 = ob_pool.tile([128, MCH, NT], F16, tag="ob")
                conv_inv(nt * NT + half, q, ob2, 0,
                         ship=[(nc.scalar, 0), (nc.sync, 1)], par_tail=True)
                ob3 = ob_pool.tile([128, MCH, NT], F16, tag="ob")
                conv_inv(nt * NT + half + q, q, ob3, 0,
                         ship=[(nc.scalar, 0), (nc.sync, 1)], par_tail=True)
            for k in FWD_LEAD.get(nt, []):
                with tc.tile_wait_until(FWD_STAMP[k]):
                    fwd_tile(k)


def _build_nc():
    nc = bacc.Bacc("TRN2", target_bir_lowering=False, debug=False)
    x = nc.dram_tensor("x", [MCH, 128, LI], F16, kind="ExternalInput").ap()
    wts16 = nc.dram_tensor("wts16", [128, N_BLKS16 * 128], F16,
                           kind="ExternalInput").ap()
    wts8 = nc.dram_tensor("wts8", [128, N_BLKS8 * 128], F8,
                          kind="ExternalInput").ap()
    out = nc.dram_tensor("out", [128, MCH, L], F16, kind="ExternalOutput").ap()
    with tile.TileContext(nc) as tc:
        _kernel_body(tc, x, wts16, wts8, out)
    nc.compile()
    return nc


def kernel(x: np.ndarray, w: np.ndarray, b: np.ndarray) -> np.ndarray:
    global LAST_RESULT
    xp = np.asarray(x, np.float32).astype(np.float16)
    xp = np.ascontiguousarray(xp).reshape(N_CORES, MCH, 128, LI)
    wts16, wts8 = _pack_weights(np.asarray(w, np.float32))

    nc = _build_nc()
    in_maps = [{"x": xp[i], "wts16": wts16, "wts8": wts8}
               for i in range(N_CORES)]
    trace = bool(int(os.environ.get("KERNEL_PROFILE", "0")))
    res = None
    last_err = None
    for attempt in range(3):
        try:
            res = run_bass_kernel_spmd(
                nc,
                in_maps,
                core_ids=list(range(N_CORES)),
                trace=trace,
            )
            break
        except Exception as e:  # transient device wedge -> retry
            last_err = e
    if res is None:
        raise last_err
    LAST_RESULT = res
    # device out: [128, 2, L] fp16, scaled by CONV_WT_SCALE
    # -> [256, H, W] f32 (unscale + bias, host-side)
    outs = []
    bias = np.asarray(b, np.float32)[:, None, None]
    for i in range(N_CORES):
        o = res.results[i]["out"].astype(np.float32)          # [128, 2, L]
        o = o.transpose(1, 0, 2).reshape(C, H, W) / CONV_WT_SCALE + bias
        outs.append(o)
    return np.stack(outs, axis=0)
